# revision 1
# baseline (speedup 1.0000x reference)
"""AIFI transformer block (attention + SpatialSILU FFN), data-parallel on 8 TRN2 cores.

Layout strategy: everything lives in "transposed" [C, N] form per sample (x's
natural layout). Per core: 32 samples, processed in pairs so weight-stationary
matmuls stream 392 columns. All weights SBUF-resident in bf16; BN folded to
per-channel affine on host; qk scale folded into W_q; v-bias folded into the
proj bias.
"""

import numpy as np
import ml_dtypes
from contextlib import ExitStack

B, C, HH, WW = 256, 256, 14, 14
N = HH * WW          # 196
HEADS, D = 8, 32
CM = 2048
NCORES = 8
EPS = 1e-5

BF16 = ml_dtypes.bfloat16

_NC_CACHE = {}




def _fin(nc, outpool, outd, s0, tiles):
    """Stage-bisect helper: touch `tiles`, write zeros to out."""
    from concourse import mybir
    f32 = mybir.dt.float32
    ot = outpool.tile([128, 2, 2, 196], f32, name=f"fin{s0}", tag="ot")
    nc.vector.memset(ot, 0.0)
    for i, t in enumerate(tiles):
        sl = t
        while len(sl.shape) > 2:
            sl = sl[:, 0]
        nc.vector.tensor_tensor(ot[:sl.shape[0], 0, 0, 0:1], ot[:sl.shape[0], 0, 0, 0:1], sl[:, 0:1], mybir.AluOpType.add)
    for s2 in range(2):
        nc.sync.dma_start(out=outd[s0 + s2].rearrange("(cc p) n -> p cc n", p=128), in_=ot[:, :, s2])

def _build(S, stage=5):
    """Build the Bass graph for S samples (must be even)."""
    import concourse.bass as bass  # noqa: F401
    import concourse.tile as tile
    from concourse import bacc, mybir

    bf = mybir.dt.bfloat16
    f32 = mybir.dt.float32
    AF = mybir.ActivationFunctionType
    OP = mybir.AluOpType

    nc = bacc.Bacc("TRN2", target_bir_lowering=False, debug=False)

    xd = nc.declare_dram_parameter("x", [S, C, N], bf, isOutput=False)
    wq_d = nc.declare_dram_parameter("wq", [C, C], bf, isOutput=False)
    wk_d = nc.declare_dram_parameter("wk", [C, C], bf, isOutput=False)
    wv_d = nc.declare_dram_parameter("wv", [C, C], bf, isOutput=False)
    wp_d = nc.declare_dram_parameter("wp", [C, C], bf, isOutput=False)
    w1_d = nc.declare_dram_parameter("w1", [C, CM], bf, isOutput=False)
    w2_d = nc.declare_dram_parameter("w2", [CM, C], bf, isOutput=False)
    id_d = nc.declare_dram_parameter("ident", [128, 128], bf, isOutput=False)
    bq_d = nc.declare_dram_parameter("bq", [128, 4], f32, isOutput=False)
    bk_d = nc.declare_dram_parameter("bk", [128, 4], f32, isOutput=False)
    b1_d = nc.declare_dram_parameter("b1", [128, 16], f32, isOutput=False)
    a1_d = nc.declare_dram_parameter("A1", [128, 2], f32, isOutput=False)
    b1p_d = nc.declare_dram_parameter("B1p", [128, 2], f32, isOutput=False)
    a2_d = nc.declare_dram_parameter("A2", [128, 2], f32, isOutput=False)
    b2p_d = nc.declare_dram_parameter("B2p", [128, 2], f32, isOutput=False)
    saw_d = nc.declare_dram_parameter("saw", [128, S], f32, isOutput=False)
    sab_d = nc.declare_dram_parameter("sab", [128, S], f32, isOutput=False)
    outd = nc.declare_dram_parameter("out", [S, C, N], f32, isOutput=True)

    NCH = [(0, 128), (128, 68)]  # token-dim chunks of 196

    with ExitStack() as ctx:
        tc = ctx.enter_context(tile.TileContext(nc))
        wpool = ctx.enter_context(tc.tile_pool(name="wpool", bufs=1))
        xpool = ctx.enter_context(tc.tile_pool(name="xpool", bufs=3))
        qkpool = ctx.enter_context(tc.tile_pool(name="qkpool", bufs=2))
        vpool = ctx.enter_context(tc.tile_pool(name="vpool", bufs=2))
        epool = ctx.enter_context(tc.tile_pool(name="epool", bufs=2))
        opool = ctx.enter_context(tc.tile_pool(name="opool", bufs=2))
        otpool = ctx.enter_context(tc.tile_pool(name="otpool", bufs=2))
        t1pool = ctx.enter_context(tc.tile_pool(name="t1pool", bufs=2))
        hpool = ctx.enter_context(tc.tile_pool(name="hpool", bufs=3))
        tmppool = ctx.enter_context(tc.tile_pool(name="tmppool", bufs=3))
        gpool = ctx.enter_context(tc.tile_pool(name="gpool", bufs=3))
        outpool = ctx.enter_context(tc.tile_pool(name="outpool", bufs=2))
        smpool = ctx.enter_context(tc.tile_pool(name="smpool", bufs=3))

        psmm = ctx.enter_context(tc.tile_pool(name="psmm", bufs=3, space="PSUM"))
        psou = ctx.enter_context(tc.tile_pool(name="psou", bufs=2, space="PSUM"))
        psT = ctx.enter_context(tc.tile_pool(name="psT", bufs=1, space="PSUM"))
        psf = ctx.enter_context(tc.tile_pool(name="psf", bufs=1, space="PSUM"))

        # ---- resident weights / params ----
        wq_sb = wpool.tile([128, 2, C], bf)
        wk_sb = wpool.tile([128, 2, C], bf)
        wv_sb = wpool.tile([128, 2, C], bf)
        wp_sb = wpool.tile([128, 2, C], bf)
        w1_sb = wpool.tile([128, 2, CM], bf)
        w2_sb = wpool.tile([128, 16, C], bf)
        id_sb = wpool.tile([128, 128], bf)
        bq_sb = wpool.tile([128, 4], f32)
        bk_sb = wpool.tile([128, 4], f32)
        b1_sb = wpool.tile([128, 16], f32)
        a1_sb = wpool.tile([128, 2], f32)
        b1p_sb = wpool.tile([128, 2], f32)
        a2_sb = wpool.tile([128, 2], f32)
        b2p_sb = wpool.tile([128, 2], f32)
        saw_sb = wpool.tile([128, S], f32)
        sab_sb = wpool.tile([128, S], f32)

        for sb, dr in (
            (wq_sb, wq_d), (wk_sb, wk_d), (wv_sb, wv_d), (wp_sb, wp_d),
        ):
            nc.sync.dma_start(out=sb, in_=dr.rearrange("(cc p) j -> p cc j", p=128))
        nc.sync.dma_start(out=w1_sb, in_=w1_d.rearrange("(cc p) j -> p cc j", p=128))
        nc.sync.dma_start(out=w2_sb, in_=w2_d.rearrange("(kc p) j -> p kc j", p=128))
        for sb, dr in (
            (id_sb, id_d), (bq_sb, bq_d), (bk_sb, bk_d), (b1_sb, b1_d),
            (a1_sb, a1_d), (b1p_sb, b1p_d), (a2_sb, a2_d), (b2p_sb, b2p_d),
            (saw_sb, saw_d), (sab_sb, sab_d),
        ):
            nc.sync.dma_start(out=sb, in_=dr.ap())

        # ---- software-pipelined per-pair emission ----
        # attn(p+1) is emitted BEFORE ffn(p): its instructions (and PSUM
        # slot requests) get higher scheduler priority, so the PE fills the
        # DVE/ACT-heavy FFN stretch of pair p with pair p+1's attention
        # matmuls instead of idling into the HAM throttle window.
        HPAIRS = [(0, 2), (4, 6), (1, 3), (5, 7)]  # equal-base pairs

        def emit_attn(pi):
            s0 = 2 * pi
            xt = xpool.tile([128, 2, 2, N], bf, name=f"xt{pi}", tag="xt")  # [p, cc, s2, n]
            for s2 in range(2):
                nc.sync.dma_start(
                    out=xt[:, :, s2],
                    in_=xd[s0 + s2].rearrange("(cc p) n -> p cc n", p=128),
                )

            # q^T, k^T : [p, ch(4 chunks of 64), s2, n]; head h at
            # (ch=h//2, base=32*(h%2)). Scores pair heads with EQUAL base —
            # matmuls with different base partitions into one PSUM tile
            # fault the exec unit.
            qt = qkpool.tile([128, 4, 2, N], bf, name=f"qt{pi}", tag="qt")
            kt = qkpool.tile([128, 4, 2, N], bf, name=f"kt{pi}", tag="kt")
            for wt, bt, dst in ((wq_sb, bq_sb, qt), (wk_sb, bk_sb, kt)):
                for ch in range(4):
                    ps1 = psmm.tile([128, 2, N], f32, name=f"psqk{pi}_{ch}", tag="mm")
                    for cc in range(2):
                        nc.tensor.matmul(
                            ps1[:64], wt[:, cc, ch * 64:(ch + 1) * 64], xt[:, cc],
                            start=(cc == 0), stop=(cc == 1),
                        )
                    nc.vector.tensor_scalar(
                        dst[:64, ch], ps1[:64], bt[:64, ch:ch + 1], None, OP.add
                    )

            # v (token-major, 33-strided heads with ones column)
            vt = vpool.tile([128, 2, 2, HEADS * 33], bf, name=f"vt{pi}", tag="vt")
            for s2 in range(2):
                for mc, (n0, nsz) in enumerate(NCH):
                    psv = psmm.tile([128, C], f32, name=f"psv{pi}_{s2}_{mc}", tag="mm")
                    for cc in range(2):
                        nc.tensor.matmul(
                            psv[:nsz], xt[:, cc, s2, n0:n0 + nsz], wv_sb[:, cc],
                            start=(cc == 0), stop=(cc == 1),
                        )
                    vv = vt[:nsz, s2, mc].rearrange("p (h e) -> p h e", e=33)
                    nc.vector.tensor_copy(
                        vv[:, :, 0:32],
                        psv[:nsz].rearrange("p (h d) -> p h d", d=32),
                    )
                    nc.gpsimd.memset(vv[:, :, 32:33], 1.0)

            # scores^T + exp (2 same-base heads per psum tile)
            expt = epool.tile([128, 2, 2, HEADS, N], bf, name=f"expt{pi}", tag="ex")
            for s2 in range(2):
                for hp, pair in enumerate(HPAIRS):
                    for mc, (m0, msz) in enumerate(NCH):
                        pss = psmm.tile([128, 2, N], f32, name=f"pss{pi}_{s2}_{hp}_{mc}", tag="mm")
                        for hh, h in enumerate(pair):
                            ch = h // 2
                            p0 = 32 * (h % 2)
                            nc.tensor.matmul(
                                pss[:msz, hh],
                                kt[p0:p0 + 32, ch, s2, m0:m0 + msz],
                                qt[p0:p0 + 32, ch, s2],
                                start=True, stop=True,
                            )
                        par, g0 = pair[0] % 2, pair[0] // 2
                        ev = expt[:msz, s2, mc].rearrange(
                            "p (g par) n -> p par g n", par=2
                        )[:, par, g0:g0 + 2]
                        nc.scalar.activation(ev, pss[:msz], AF.Exp)

            # o = attn @ v (colsum via ones col), normalize on evac
            o_sb = opool.tile([128, 2, 2, C], bf, name=f"o{pi}", tag="o")
            for s2 in range(2):
                for ni, (n0, nsz) in enumerate(NCH):
                    pso = psou.tile([128, HEADS * 33], f32, name=f"pso{pi}_{s2}_{ni}", tag="ou")
                    for h in range(HEADS):
                        for mc, (m0, msz) in enumerate(NCH):
                            nc.tensor.matmul(
                                pso[:nsz, h * 33:(h + 1) * 33],
                                expt[:msz, s2, mc, h, n0:n0 + nsz],
                                vt[:msz, s2, mc, h * 33:(h + 1) * 33],
                                start=(mc == 0), stop=(mc == 1),
                            )
                    pv = pso[:nsz].rearrange("p (h e) -> p h e", e=33)
                    rc = smpool.tile([128, HEADS], f32, name=f"rc{pi}_{s2}_{ni}", tag="rc")
                    nc.vector.reciprocal(rc[:nsz], pv[:, :, 32])
                    ob = o_sb[:nsz, s2, ni].rearrange("p (h d) -> p h d", d=32)
                    rcb = bass.AP(
                        tensor=rc.tensor, offset=rc.offset,
                        ap=[list(rc.ap[0])[:1] + [nsz]] + [list(rc.ap[1])] + [[0, 32]],
                    )
                    nc.vector.tensor_tensor(ob, pv[:, :, 0:32], rcb, OP.mult)

            # transpose o -> oT [c, n]
            oT_sb = otpool.tile([128, 2, 2, N], bf, name=f"oT{pi}", tag="oT")
            for cc in range(2):
                psot = psT.tile([128, 2, N], bf, name=f"psot{pi}_{cc}", tag="pst")
                for s2 in range(2):
                    for ni, (n0, nsz) in enumerate(NCH):
                        nc.tensor.transpose(
                            psot[:, s2, n0:n0 + nsz],
                            o_sb[:nsz, s2, ni, cc * 128:(cc + 1) * 128],
                            id_sb[:nsz, :nsz],
                        )
                nc.vector.tensor_copy(oT_sb[:, cc], psot)

            # proj + residual + RepBN1
            t1_sb = t1pool.tile([128, 2, 2, N], bf, name=f"t1{pi}", tag="t1")
            for jc in range(2):
                psp = psmm.tile([128, 2, N], f32, name=f"psp{pi}_{jc}", tag="mm")
                for cc in range(2):
                    nc.tensor.matmul(
                        psp, wp_sb[:, cc, jc * 128:(jc + 1) * 128], oT_sb[:, cc],
                        start=(cc == 0), stop=(cc == 1),
                    )
                vadd = tmppool.tile([128, 2, N], bf, name=f"vadd{pi}_{jc}", tag="vadd")
                nc.vector.tensor_tensor(vadd, psp, xt[:, jc], OP.add)
                nc.vector.tensor_scalar(
                    t1_sb[:, jc], vadd, a1_sb[:, jc:jc + 1], b1p_sb[:, jc:jc + 1],
                    OP.mult, OP.add,
                )
            return t1_sb

        def emit_ffn(pi, t1_sb):
            s0 = 2 * pi
            psfj = [
                psf.tile([128, 2, N], f32, name=f"psf{pi}_{jc}", tag=f"f{jc}")
                for jc in range(2)
            ]
            # 4-kc slabs: psum evac stays per-kc, but the SILU chain runs
            # on [128, 4*392] slabs (4x fewer DVE/ACT dispatches)
            for sl in range(4):
                h_all = hpool.tile([128, 4, 2, N], bf, name=f"hs{pi}_{sl}", tag="hs")
                for k4 in range(4):
                    kc = sl * 4 + k4
                    psh = psmm.tile([128, 2, N], f32, name=f"psh{pi}_{kc}", tag="mm")
                    for cc in range(2):
                        nc.tensor.matmul(
                            psh, w1_sb[:, cc, kc * 128:(kc + 1) * 128], t1_sb[:, cc],
                            start=(cc == 0), stop=(cc == 1),
                        )
                    if kc % 2 == 0:
                        nc.vector.tensor_scalar(
                            h_all[:, k4], psh, b1_sb[:, kc:kc + 1], None, OP.add
                        )
                    else:
                        nc.scalar.activation(
                            h_all[:, k4], psh, AF.Identity, bias=b1_sb[:, kc:kc + 1]
                        )
                t0 = tmppool.tile([128, 4, 2, N], bf, name=f"t0{pi}_{sl}", tag="t0")
                sgm = tmppool.tile([128, 4, 2, N], bf, name=f"sg{pi}_{sl}", tag="sg")
                g_all = gpool.tile([128, 4, 2, N], bf, name=f"g{pi}_{sl}", tag="g")
                for s2 in range(2):
                    nc.vector.tensor_scalar(
                        t0[:, :, s2], h_all[:, :, s2],
                        saw_sb[:, s0 + s2:s0 + s2 + 1],
                        sab_sb[:, s0 + s2:s0 + s2 + 1],
                        OP.mult, OP.add,
                    )
                nc.vector.tensor_tensor(t0, t0, h_all, OP.mult)
                nc.scalar.activation(sgm, t0, AF.Sigmoid)
                nc.vector.tensor_tensor(g_all, sgm, h_all, OP.mult)
                for k4 in range(4):
                    kc = sl * 4 + k4
                    for jc in range(2):
                        nc.tensor.matmul(
                            psfj[jc], w2_sb[:, kc, jc * 128:(jc + 1) * 128],
                            g_all[:, k4],
                            start=(kc == 0), stop=(kc == 15),
                        )

            # residual + RepBN2 -> out
            ot = outpool.tile([128, 2, 2, N], f32, name=f"ot{pi}", tag="ot")
            for jc in range(2):
                v2 = tmppool.tile([128, 2, N], f32, name=f"v2{pi}_{jc}", tag="v2")
                nc.vector.tensor_tensor(v2, psfj[jc], t1_sb[:, jc], OP.add)
                nc.vector.tensor_scalar(
                    ot[:, jc], v2, a2_sb[:, jc:jc + 1], b2p_sb[:, jc:jc + 1],
                    OP.mult, OP.add,
                )
            for s2 in range(2):
                nc.sync.dma_start(
                    out=outd[s0 + s2].rearrange("(cc p) n -> p cc n", p=128),
                    in_=ot[:, :, s2],
                )

        npairs = S // 2
        t1_prev = emit_attn(0)
        for pi in range(npairs):
            t1_next = emit_attn(pi + 1) if pi + 1 < npairs else None
            emit_ffn(pi, t1_prev)
            t1_prev = t1_next

    return nc


def _get_nc(S):
    if S not in _NC_CACHE:
        _NC_CACHE[S] = _build(S)
    return _NC_CACHE[S]


def _prep_inputs(inputs, S):
    """Host-side preprocessing + sharding. Returns in_maps (len NCORES)."""
    x = np.asarray(inputs["x"], np.float32).reshape(B, C, N)
    W_qkv = np.asarray(inputs["W_qkv"], np.float32)
    b_qkv = np.asarray(inputs["b_qkv"], np.float32)
    W_proj = np.asarray(inputs["W_proj"], np.float32)
    b_proj = np.asarray(inputs["b_proj"], np.float32)
    W1 = np.asarray(inputs["W1"], np.float32)
    b1 = np.asarray(inputs["b1"], np.float32)
    W2 = np.asarray(inputs["W2"], np.float32)
    b2 = np.asarray(inputs["b2"], np.float32)
    sa_w = np.asarray(inputs["sa_w"], np.float32)
    sa_b = np.asarray(inputs["sa_b"], np.float32)

    def g(name):
        return np.asarray(inputs[name], np.float32)

    scale = D ** -0.5
    Wq = W_qkv[:, 0:C] * scale
    bq = b_qkv[0:C] * scale
    Wk = W_qkv[:, C:2 * C]
    bk = b_qkv[C:2 * C]
    Wv = W_qkv[:, 2 * C:3 * C]
    bv = b_qkv[2 * C:3 * C]
    b_proj_eff = b_proj + bv @ W_proj

    s1 = g("gamma1") / np.sqrt(g("rv1") + EPS)
    A1 = np.float32(g("alpha1")) + s1
    B1 = g("beta1") - g("rm1") * s1
    B1p = A1 * b_proj_eff + B1
    s2v = g("gamma2") / np.sqrt(g("rv2") + EPS)
    A2 = np.float32(g("alpha2")) + s2v
    B2 = g("beta2") - g("rm2") * s2v
    B2p = A2 * b2 + B2

    def v2sb(v):  # [256] -> [128, 2]
        return np.ascontiguousarray(v.reshape(2, 128).T)

    def v2sb96(v):  # [256] -> [128, 4], chunks of 64 zero-padded
        o = np.zeros((128, 4), np.float32)
        for ch in range(4):
            o[0:64, ch] = v[ch * 64:(ch + 1) * 64]
        return o

    common = {
        "wq": Wq.astype(BF16), "wk": Wk.astype(BF16), "wv": Wv.astype(BF16),
        "wp": W_proj.astype(BF16), "w1": W1.astype(BF16), "w2": W2.astype(BF16),
        "ident": np.eye(128, dtype=BF16),
        "bq": v2sb96(bq), "bk": v2sb96(bk),
        "b1": np.ascontiguousarray(b1.reshape(16, 128).T),
        "A1": v2sb(A1), "B1p": v2sb(B1p), "A2": v2sb(A2), "B2p": v2sb(B2p),
    }
    xb = x.astype(BF16)
    in_maps = []
    for c in range(NCORES):
        sl = slice(c * S, (c + 1) * S)
        m = dict(common)
        m["x"] = np.ascontiguousarray(xb[sl])
        m["saw"] = np.ascontiguousarray(
            np.broadcast_to(sa_w[sl][None, :], (128, S)).astype(np.float32))
        m["sab"] = np.ascontiguousarray(
            np.broadcast_to(sa_b[sl][None, :], (128, S)).astype(np.float32))
        in_maps.append(m)
    return in_maps


_LDW_OPT = [False]


def _patch_ldw_opt():
    if _LDW_OPT[0]:
        return
    import concourse.bass_utils as bu
    orig = bu.run_command

    def patched(cmd, **kw):
        cmd = ["--enable-ldw-opt=true" if c == "--enable-ldw-opt=false" else c
               for c in cmd]
        return orig(cmd, **kw)

    bu.run_command = patched
    _LDW_OPT[0] = True


def run(trace=False, **inputs):
    """Returns (out [B,C,H,W] f32, exec_time_ns or None)."""
    from concourse.bass_utils import run_bass_kernel_spmd

    S = B // NCORES
    nc = _get_nc(S)
    if not nc.is_finalized():
        nc.finalize()
    in_maps = _prep_inputs(inputs, S)
    res = run_bass_kernel_spmd(nc, in_maps, core_ids=list(range(NCORES)), trace=trace)
    outs = [np.asarray(r["out"], np.float32) for r in res.results]
    out = np.concatenate(outs, axis=0).reshape(B, C, HH, WW)
    return out, res.exec_time_ns


def kernel(**inputs):
    return run(trace=False, **inputs)[0]



# revision 8
# speedup vs baseline: 1.1320x; 1.1320x over previous
"""AIFI transformer block (attention + SpatialSILU FFN), data-parallel on 8 TRN2 cores.

Layout strategy: everything lives in "transposed" [C, N] form per sample (x's
natural layout). Per core: 32 samples, processed in pairs so weight-stationary
matmuls stream 392 columns. All weights SBUF-resident in bf16; BN folded to
per-channel affine on host; qk scale folded into W_q; v-bias folded into the
proj bias.
"""

import numpy as np
import ml_dtypes
from contextlib import ExitStack

B, C, HH, WW = 256, 256, 14, 14
N = HH * WW          # 196
HEADS, D = 8, 32
CM = 2048
NCORES = 8
EPS = 1e-5

BF16 = ml_dtypes.bfloat16

_NC_CACHE = {}




def _fin(nc, outpool, outd, s0, tiles):
    """Stage-bisect helper: touch `tiles`, write zeros to out."""
    from concourse import mybir
    f32 = mybir.dt.float32
    ot = outpool.tile([128, 2, 2, 196], f32, name=f"fin{s0}", tag="ot")
    nc.vector.memset(ot, 0.0)
    for i, t in enumerate(tiles):
        sl = t
        while len(sl.shape) > 2:
            sl = sl[:, 0]
        nc.vector.tensor_tensor(ot[:sl.shape[0], 0, 0, 0:1], ot[:sl.shape[0], 0, 0, 0:1], sl[:, 0:1], mybir.AluOpType.add)
    for s2 in range(2):
        nc.sync.dma_start(out=outd[s0 + s2].rearrange("(cc p) n -> p cc n", p=128), in_=ot[:, :, s2])

def _build(S, stage=5):
    """Build the Bass graph for S samples (must be even)."""
    import concourse.bass as bass  # noqa: F401
    import concourse.tile as tile
    from concourse import bacc, mybir

    bf = mybir.dt.bfloat16
    f32 = mybir.dt.float32
    AF = mybir.ActivationFunctionType
    OP = mybir.AluOpType

    nc = bacc.Bacc("TRN2", target_bir_lowering=False, debug=False)

    xd = nc.declare_dram_parameter("x", [S, C, N], bf, isOutput=False)
    wq_d = nc.declare_dram_parameter("wq", [C, C], bf, isOutput=False)
    wk_d = nc.declare_dram_parameter("wk", [C, C], bf, isOutput=False)
    wv_d = nc.declare_dram_parameter("wv", [C, C], bf, isOutput=False)
    wp_d = nc.declare_dram_parameter("wp", [C, C], bf, isOutput=False)
    w1_d = nc.declare_dram_parameter("w1", [C, CM], bf, isOutput=False)
    w2_d = nc.declare_dram_parameter("w2", [CM, C], bf, isOutput=False)
    id_d = nc.declare_dram_parameter("ident", [128, 128], bf, isOutput=False)
    bq_d = nc.declare_dram_parameter("bq", [128, 4], f32, isOutput=False)
    bk_d = nc.declare_dram_parameter("bk", [128, 4], f32, isOutput=False)
    b1_d = nc.declare_dram_parameter("b1", [128, 16], f32, isOutput=False)
    a1_d = nc.declare_dram_parameter("A1", [128, 2], f32, isOutput=False)
    b1p_d = nc.declare_dram_parameter("B1p", [128, 2], f32, isOutput=False)
    a2_d = nc.declare_dram_parameter("A2", [128, 2], f32, isOutput=False)
    b2p_d = nc.declare_dram_parameter("B2p", [128, 2], f32, isOutput=False)
    saw_d = nc.declare_dram_parameter("saw", [128, S], f32, isOutput=False)
    sab_d = nc.declare_dram_parameter("sab", [128, S], f32, isOutput=False)
    outd = nc.declare_dram_parameter("out", [S, C, N], f32, isOutput=True)

    NCH = [(0, 128), (128, 68)]  # token-dim chunks of 196

    with ExitStack() as ctx:
        tc = ctx.enter_context(tile.TileContext(nc))
        wpool = ctx.enter_context(tc.tile_pool(name="wpool", bufs=1))
        xpool = ctx.enter_context(tc.tile_pool(name="xpool", bufs=3))
        qkpool = ctx.enter_context(tc.tile_pool(name="qkpool", bufs=2))
        vpool = ctx.enter_context(tc.tile_pool(name="vpool", bufs=2))
        epool = ctx.enter_context(tc.tile_pool(name="epool", bufs=2))
        opool = ctx.enter_context(tc.tile_pool(name="opool", bufs=2))
        otpool = ctx.enter_context(tc.tile_pool(name="otpool", bufs=2))
        t1pool = ctx.enter_context(tc.tile_pool(name="t1pool", bufs=2))
        hpool = ctx.enter_context(tc.tile_pool(name="hpool", bufs=3))
        tmppool = ctx.enter_context(tc.tile_pool(name="tmppool", bufs=3))
        gpool = ctx.enter_context(tc.tile_pool(name="gpool", bufs=3))
        outpool = ctx.enter_context(tc.tile_pool(name="outpool", bufs=2))
        smpool = ctx.enter_context(tc.tile_pool(name="smpool", bufs=3))

        psmm = ctx.enter_context(tc.tile_pool(name="psmm", bufs=3, space="PSUM"))
        psou = ctx.enter_context(tc.tile_pool(name="psou", bufs=2, space="PSUM"))
        psT = ctx.enter_context(tc.tile_pool(name="psT", bufs=1, space="PSUM"))
        psf = ctx.enter_context(tc.tile_pool(name="psf", bufs=1, space="PSUM"))

        # ---- resident weights / params ----
        wq_sb = wpool.tile([128, 2, C], bf)
        wk_sb = wpool.tile([128, 2, C], bf)
        wv_sb = wpool.tile([128, 2, C], bf)
        wp_sb = wpool.tile([128, 2, C], bf)
        w1_sb = wpool.tile([128, 2, CM], bf)
        w2_sb = wpool.tile([128, 16, C], bf)
        id_sb = wpool.tile([128, 128], bf)
        bq_sb = wpool.tile([128, 4], f32)
        bk_sb = wpool.tile([128, 4], f32)
        b1_sb = wpool.tile([128, 16], f32)
        a1_sb = wpool.tile([128, 2], f32)
        b1p_sb = wpool.tile([128, 2], f32)
        a2_sb = wpool.tile([128, 2], f32)
        b2p_sb = wpool.tile([128, 2], f32)
        saw_sb = wpool.tile([128, S], f32)
        sab_sb = wpool.tile([128, S], f32)

        for sb, dr in (
            (wq_sb, wq_d), (wk_sb, wk_d), (wv_sb, wv_d), (wp_sb, wp_d),
        ):
            nc.sync.dma_start(out=sb, in_=dr.rearrange("(cc p) j -> p cc j", p=128))
        nc.sync.dma_start(out=w1_sb, in_=w1_d.rearrange("(cc p) j -> p cc j", p=128))
        nc.sync.dma_start(out=w2_sb, in_=w2_d.rearrange("(kc p) j -> p kc j", p=128))
        for sb, dr in (
            (id_sb, id_d), (bq_sb, bq_d), (bk_sb, bk_d), (b1_sb, b1_d),
            (a1_sb, a1_d), (b1p_sb, b1p_d), (a2_sb, a2_d), (b2p_sb, b2p_d),
            (saw_sb, saw_d), (sab_sb, sab_d),
        ):
            nc.sync.dma_start(out=sb, in_=dr.ap())

        # ---- software-pipelined per-pair emission ----
        # attn(p+1) is emitted BEFORE ffn(p): its instructions (and PSUM
        # slot requests) get higher scheduler priority, so the PE fills the
        # DVE/ACT-heavy FFN stretch of pair p with pair p+1's attention
        # matmuls instead of idling into the HAM throttle window.
        HPAIRS = [(0, 2), (4, 6), (1, 3), (5, 7)]  # equal-base pairs

        def emit_attn(pi):
            s0 = 2 * pi
            xt = xpool.tile([128, 2, 2, N], bf, name=f"xt{pi}", tag="xt")  # [p, cc, s2, n]
            for s2 in range(2):
                nc.sync.dma_start(
                    out=xt[:, :, s2],
                    in_=xd[s0 + s2].rearrange("(cc p) n -> p cc n", p=128),
                )

            # q^T, k^T : [p, ch(4 chunks of 64), s2, n]; head h at
            # (ch=h//2, base=32*(h%2)). Scores pair heads with EQUAL base —
            # matmuls with different base partitions into one PSUM tile
            # fault the exec unit.
            qt = qkpool.tile([128, 4, 2, N], bf, name=f"qt{pi}", tag="qt")
            kt = qkpool.tile([128, 4, 2, N], bf, name=f"kt{pi}", tag="kt")
            for wt, bt, dst in ((wq_sb, bq_sb, qt), (wk_sb, bk_sb, kt)):
                for ch in range(4):
                    ps1 = psmm.tile([128, 2, N], f32, name=f"psqk{pi}_{ch}", tag="mm")
                    for cc in range(2):
                        nc.tensor.matmul(
                            ps1[:64], wt[:, cc, ch * 64:(ch + 1) * 64], xt[:, cc],
                            start=(cc == 0), stop=(cc == 1),
                        )
                    nc.scalar.activation(
                        dst[:64, ch], ps1[:64], AF.Identity, bias=bt[:64, ch:ch + 1]
                    )

            # v (token-major, 33-strided heads with ones column)
            vt = vpool.tile([128, 2, 2, HEADS * 33], bf, name=f"vt{pi}", tag="vt")
            for s2 in range(2):
                for mc, (n0, nsz) in enumerate(NCH):
                    psv = psmm.tile([128, C], f32, name=f"psv{pi}_{s2}_{mc}", tag="mm")
                    for cc in range(2):
                        nc.tensor.matmul(
                            psv[:nsz], xt[:, cc, s2, n0:n0 + nsz], wv_sb[:, cc],
                            start=(cc == 0), stop=(cc == 1),
                        )
                    vv = vt[:nsz, s2, mc].rearrange("p (h e) -> p h e", e=33)
                    nc.vector.tensor_copy(
                        vv[:, :, 0:32],
                        psv[:nsz].rearrange("p (h d) -> p h d", d=32),
                    )
                    nc.gpsimd.memset(vv[:, :, 32:33], 1.0)

            # scores^T + exp (2 same-base heads per psum tile)
            expt = epool.tile([128, 2, 2, HEADS, N], bf, name=f"expt{pi}", tag="ex")
            for s2 in range(2):
                for hp, pair in enumerate(HPAIRS):
                    for mc, (m0, msz) in enumerate(NCH):
                        pss = psmm.tile([128, 2, N], f32, name=f"pss{pi}_{s2}_{hp}_{mc}", tag="mm")
                        for hh, h in enumerate(pair):
                            ch = h // 2
                            p0 = 32 * (h % 2)
                            nc.tensor.matmul(
                                pss[:msz, hh],
                                kt[p0:p0 + 32, ch, s2, m0:m0 + msz],
                                qt[p0:p0 + 32, ch, s2],
                                start=True, stop=True,
                            )
                        par, g0 = pair[0] % 2, pair[0] // 2
                        ev = expt[:msz, s2, mc].rearrange(
                            "p (g par) n -> p par g n", par=2
                        )[:, par, g0:g0 + 2]
                        nc.scalar.activation(ev, pss[:msz], AF.Exp)

            # o = attn @ v (colsum via ones col), normalize on evac
            o_sb = opool.tile([128, 2, 2, C], bf, name=f"o{pi}", tag="o")
            for s2 in range(2):
                for ni, (n0, nsz) in enumerate(NCH):
                    pso = psou.tile([128, HEADS * 33], f32, name=f"pso{pi}_{s2}_{ni}", tag="ou")
                    for h in range(HEADS):
                        for mc, (m0, msz) in enumerate(NCH):
                            nc.tensor.matmul(
                                pso[:nsz, h * 33:(h + 1) * 33],
                                expt[:msz, s2, mc, h, n0:n0 + nsz],
                                vt[:msz, s2, mc, h * 33:(h + 1) * 33],
                                start=(mc == 0), stop=(mc == 1),
                            )
                    pv = pso[:nsz].rearrange("p (h e) -> p h e", e=33)
                    rc = smpool.tile([128, HEADS], f32, name=f"rc{pi}_{s2}_{ni}", tag="rc")
                    nc.vector.reciprocal(rc[:nsz], pv[:, :, 32])
                    ob = o_sb[:nsz, s2, ni].rearrange("p (h d) -> p h d", d=32)
                    rcb = bass.AP(
                        tensor=rc.tensor, offset=rc.offset,
                        ap=[list(rc.ap[0])[:1] + [nsz]] + [list(rc.ap[1])] + [[0, 32]],
                    )
                    nc.vector.tensor_tensor(ob, pv[:, :, 0:32], rcb, OP.mult)

            # transpose o -> oT [c, n]
            oT_sb = otpool.tile([128, 2, 2, N], bf, name=f"oT{pi}", tag="oT")
            for cc in range(2):
                psot = psT.tile([128, 2, N], bf, name=f"psot{pi}_{cc}", tag="pst")
                for s2 in range(2):
                    for ni, (n0, nsz) in enumerate(NCH):
                        nc.tensor.transpose(
                            psot[:, s2, n0:n0 + nsz],
                            o_sb[:nsz, s2, ni, cc * 128:(cc + 1) * 128],
                            id_sb[:nsz, :nsz],
                        )
                nc.vector.tensor_copy(oT_sb[:, cc], psot)

            # proj + residual + RepBN1:  t1 = A1*psp + (A1*xt + B1p)
            xb = tmppool.tile([128, 2, 2, N], bf, name=f"xb{pi}", tag="xb")
            for jc in range(2):
                nc.vector.tensor_scalar(
                    xb[:, jc], xt[:, jc], a1_sb[:, jc:jc + 1], b1p_sb[:, jc:jc + 1],
                    OP.mult, OP.add,
                )
            t1_sb = t1pool.tile([128, 2, 2, N], bf, name=f"t1{pi}", tag="t1")
            for jc in range(2):
                psp = psmm.tile([128, 2, N], f32, name=f"psp{pi}_{jc}", tag="mm")
                for cc in range(2):
                    nc.tensor.matmul(
                        psp, wp_sb[:, cc, jc * 128:(jc + 1) * 128], oT_sb[:, cc],
                        start=(cc == 0), stop=(cc == 1),
                    )
                nc.vector.scalar_tensor_tensor(
                    t1_sb[:, jc], psp, a1_sb[:, jc:jc + 1], xb[:, jc],
                    OP.mult, OP.add,
                )
            return t1_sb

        def emit_ffn(pi, t1_sb):
            s0 = 2 * pi
            psfj = [
                psf.tile([128, 2, N], f32, name=f"psf{pi}_{jc}", tag=f"f{jc}")
                for jc in range(2)
            ]
            # t1b = A2*t1 + B2p, consumed by the fused output evac
            t1b = tmppool.tile([128, 2, 2, N], bf, name=f"t1b{pi}", tag="t1b")
            for jc in range(2):
                nc.vector.tensor_scalar(
                    t1b[:, jc], t1_sb[:, jc], a2_sb[:, jc:jc + 1],
                    b2p_sb[:, jc:jc + 1], OP.mult, OP.add,
                )
            # 4-kc slabs; SILU via tanh (same ACT table set as exp):
            #   u = (h+c)*h ; th = tanh(saw2*u) ; g' = (th+1)*h ; W2 pre-halved
            for sl in range(4):
                h_all = hpool.tile([128, 4, 2, N], bf, name=f"hs{pi}_{sl}", tag="hs")
                for k4 in range(4):
                    kc = sl * 4 + k4
                    psh = psmm.tile([128, 2, N], f32, name=f"psh{pi}_{kc}", tag="mm")
                    for cc in range(2):
                        nc.tensor.matmul(
                            psh, w1_sb[:, cc, kc * 128:(kc + 1) * 128], t1_sb[:, cc],
                            start=(cc == 0), stop=(cc == 1),
                        )
                    if kc % 4 == 0:
                        nc.vector.tensor_scalar(
                            h_all[:, k4], psh, b1_sb[:, kc:kc + 1], None, OP.add
                        )
                    else:
                        nc.scalar.activation(
                            h_all[:, k4], psh, AF.Identity, bias=b1_sb[:, kc:kc + 1]
                        )
                u = tmppool.tile([128, 4, 2, N], bf, name=f"u{pi}_{sl}", tag="u")
                th = tmppool.tile([128, 4, 2, N], bf, name=f"th{pi}_{sl}", tag="th")
                g_all = gpool.tile([128, 4, 2, N], bf, name=f"g{pi}_{sl}", tag="g")
                for s2 in range(2):
                    nc.vector.scalar_tensor_tensor(
                        u[:, :, s2], h_all[:, :, s2],
                        sab_sb[:, s0 + s2:s0 + s2 + 1], h_all[:, :, s2],
                        OP.add, OP.mult,
                    )
                    nc.scalar.activation(
                        th[:, :, s2], u[:, :, s2], AF.Tanh,
                        scale=saw_sb[:, s0 + s2:s0 + s2 + 1],
                    )
                nc.vector.scalar_tensor_tensor(
                    g_all, th, 1.0, h_all, OP.add, OP.mult,
                )
                for k4 in range(4):
                    kc = sl * 4 + k4
                    for jc in range(2):
                        nc.tensor.matmul(
                            psfj[jc], w2_sb[:, kc, jc * 128:(jc + 1) * 128],
                            g_all[:, k4],
                            start=(kc == 0), stop=(kc == 15),
                        )

            # residual + RepBN2 -> out:  ot = A2*psf + t1b
            ot = outpool.tile([128, 2, 2, N], f32, name=f"ot{pi}", tag="ot")
            for jc in range(2):
                nc.vector.scalar_tensor_tensor(
                    ot[:, jc], psfj[jc], a2_sb[:, jc:jc + 1], t1b[:, jc],
                    OP.mult, OP.add,
                )
            for s2 in range(2):
                nc.sync.dma_start(
                    out=outd[s0 + s2].rearrange("(cc p) n -> p cc n", p=128),
                    in_=ot[:, :, s2],
                )

        npairs = S // 2
        t1_prev = emit_attn(0)
        for pi in range(npairs):
            t1_next = emit_attn(pi + 1) if pi + 1 < npairs else None
            emit_ffn(pi, t1_prev)
            t1_prev = t1_next

    return nc


def _get_nc(S):
    if S not in _NC_CACHE:
        _NC_CACHE[S] = _build(S)
    return _NC_CACHE[S]


def _prep_inputs(inputs, S):
    """Host-side preprocessing + sharding. Returns in_maps (len NCORES)."""
    x = np.asarray(inputs["x"], np.float32).reshape(B, C, N)
    W_qkv = np.asarray(inputs["W_qkv"], np.float32)
    b_qkv = np.asarray(inputs["b_qkv"], np.float32)
    W_proj = np.asarray(inputs["W_proj"], np.float32)
    b_proj = np.asarray(inputs["b_proj"], np.float32)
    W1 = np.asarray(inputs["W1"], np.float32)
    b1 = np.asarray(inputs["b1"], np.float32)
    W2 = np.asarray(inputs["W2"], np.float32)
    b2 = np.asarray(inputs["b2"], np.float32)
    sa_w = np.asarray(inputs["sa_w"], np.float32)
    sa_b = np.asarray(inputs["sa_b"], np.float32)

    def g(name):
        return np.asarray(inputs[name], np.float32)

    scale = D ** -0.5
    Wq = W_qkv[:, 0:C] * scale
    bq = b_qkv[0:C] * scale
    Wk = W_qkv[:, C:2 * C]
    bk = b_qkv[C:2 * C]
    Wv = W_qkv[:, 2 * C:3 * C]
    bv = b_qkv[2 * C:3 * C]
    b_proj_eff = b_proj + bv @ W_proj

    s1 = g("gamma1") / np.sqrt(g("rv1") + EPS)
    A1 = np.float32(g("alpha1")) + s1
    B1 = g("beta1") - g("rm1") * s1
    B1p = A1 * b_proj_eff + B1
    s2v = g("gamma2") / np.sqrt(g("rv2") + EPS)
    A2 = np.float32(g("alpha2")) + s2v
    B2 = g("beta2") - g("rm2") * s2v
    B2p = A2 * b2 + B2

    # sigmoid(w) = (1+tanh(w/2))/2 : tanh lives in the same ACT table set as
    # exp, so the kernel never reloads activation tables. Fold the /2 into
    # sa_w (tanh arg) and W2 (output):  g = h*sigmoid((sa_w*h+sa_b)*h)
    #   u  = (h + c)*h          with c = sa_b/sa_w
    #   th = tanh(saw2 * u)     with saw2 = sa_w/2   (ACT scale)
    #   g' = (th + 1)*h ;  f = g' @ (W2/2)
    saw2 = sa_w * 0.5
    c_silu = np.divide(sa_b, sa_w, out=np.zeros_like(sa_b),
                       where=(sa_w != 0)).astype(np.float32)

    def v2sb(v):  # [256] -> [128, 2]
        return np.ascontiguousarray(v.reshape(2, 128).T)

    def v2sb96(v):  # [256] -> [128, 4], chunks of 64 zero-padded
        o = np.zeros((128, 4), np.float32)
        for ch in range(4):
            o[0:64, ch] = v[ch * 64:(ch + 1) * 64]
        return o

    common = {
        "wq": Wq.astype(BF16), "wk": Wk.astype(BF16), "wv": Wv.astype(BF16),
        "wp": W_proj.astype(BF16), "w1": W1.astype(BF16),
        "w2": (W2 * 0.5).astype(BF16),
        "ident": np.eye(128, dtype=BF16),
        "bq": v2sb96(bq), "bk": v2sb96(bk),
        "b1": np.ascontiguousarray(b1.reshape(16, 128).T),
        "A1": v2sb(A1), "B1p": v2sb(B1p), "A2": v2sb(A2), "B2p": v2sb(B2p),
    }
    xb = x.astype(BF16)
    in_maps = []
    for c in range(NCORES):
        sl = slice(c * S, (c + 1) * S)
        m = dict(common)
        m["x"] = np.ascontiguousarray(xb[sl])
        m["saw"] = np.ascontiguousarray(
            np.broadcast_to(saw2[sl][None, :], (128, S)).astype(np.float32))
        m["sab"] = np.ascontiguousarray(
            np.broadcast_to(c_silu[sl][None, :], (128, S)).astype(np.float32))
        in_maps.append(m)
    return in_maps


_LDW_OPT = [False]


def _patch_ldw_opt():
    if _LDW_OPT[0]:
        return
    import concourse.bass_utils as bu
    orig = bu.run_command

    def patched(cmd, **kw):
        cmd = ["--enable-ldw-opt=true" if c == "--enable-ldw-opt=false" else c
               for c in cmd]
        return orig(cmd, **kw)

    bu.run_command = patched
    _LDW_OPT[0] = True


def run(trace=False, **inputs):
    """Returns (out [B,C,H,W] f32, exec_time_ns or None)."""
    from concourse.bass_utils import run_bass_kernel_spmd

    S = B // NCORES
    nc = _get_nc(S)
    if not nc.is_finalized():
        nc.finalize()
    in_maps = _prep_inputs(inputs, S)
    res = run_bass_kernel_spmd(nc, in_maps, core_ids=list(range(NCORES)), trace=trace)
    outs = [np.asarray(r["out"], np.float32) for r in res.results]
    out = np.concatenate(outs, axis=0).reshape(B, C, HH, WW)
    return out, res.exec_time_ns


def kernel(**inputs):
    return run(trace=False, **inputs)[0]



# revision 21
# speedup vs baseline: 1.2796x; 1.1303x over previous
"""AIFI transformer block (attention + SpatialSILU FFN), data-parallel on 8 TRN2 cores.

Layout strategy: everything lives in "transposed" [C, N] form per sample (x's
natural layout). Per core: 32 samples, processed in pairs so weight-stationary
matmuls stream 392 columns. All weights SBUF-resident in bf16; BN folded to
per-channel affine on host; qk scale folded into W_q; v-bias folded into the
proj bias.
"""

import numpy as np
import ml_dtypes
from contextlib import ExitStack

B, C, HH, WW = 256, 256, 14, 14
N = HH * WW          # 196
HEADS, D = 8, 32
CM = 2048
NCORES = 8
EPS = 1e-5

BF16 = ml_dtypes.bfloat16

_NC_CACHE = {}




def _fin(nc, outpool, outd, s0, tiles):
    """Stage-bisect helper: touch `tiles`, write zeros to out."""
    from concourse import mybir
    f32 = mybir.dt.float32
    ot = outpool.tile([128, 2, 2, 196], f32, name=f"fin{s0}", tag="ot")
    nc.vector.memset(ot, 0.0)
    for i, t in enumerate(tiles):
        sl = t
        while len(sl.shape) > 2:
            sl = sl[:, 0]
        nc.vector.tensor_tensor(ot[:sl.shape[0], 0, 0, 0:1], ot[:sl.shape[0], 0, 0, 0:1], sl[:, 0:1], mybir.AluOpType.add)
    for s2 in range(2):
        nc.sync.dma_start(out=outd[s0 + s2].rearrange("(cc p) n -> p cc n", p=128), in_=ot[:, :, s2])

def _build(S, has_c=False, stage=5):
    """Build the Bass graph for S samples (must be even).

    has_c: general path for sa_b != 0 (u = (h+c)*h via STT); the fast path
    (sa_b == 0, the reference's setup) uses u = h*h as a single TT.
    """
    import concourse.bass as bass  # noqa: F401
    import concourse.tile as tile
    from concourse import bacc, mybir

    bf = mybir.dt.bfloat16
    f32 = mybir.dt.float32
    AF = mybir.ActivationFunctionType
    OP = mybir.AluOpType

    nc = bacc.Bacc("TRN2", target_bir_lowering=False, debug=False)

    xd = nc.declare_dram_parameter("x", [S, C, N], bf, isOutput=False)
    wq_d = nc.declare_dram_parameter("wq", [C, C], bf, isOutput=False)
    wk_d = nc.declare_dram_parameter("wk", [C, C], bf, isOutput=False)
    wv_d = nc.declare_dram_parameter("wv", [C, C], bf, isOutput=False)
    wp_d = nc.declare_dram_parameter("wp", [C, C], bf, isOutput=False)
    w1_d = nc.declare_dram_parameter("w1", [C, CM], bf, isOutput=False)
    w2_d = nc.declare_dram_parameter("w2", [CM, C], bf, isOutput=False)
    id_d = nc.declare_dram_parameter("ident", [128, 128], bf, isOutput=False)
    bq_d = nc.declare_dram_parameter("bq", [128, 2], f32, isOutput=False)
    bk_d = nc.declare_dram_parameter("bk", [128, 2], f32, isOutput=False)
    b1_d = nc.declare_dram_parameter("b1", [128, 16], f32, isOutput=False)
    a1_d = nc.declare_dram_parameter("A1", [128, 2], f32, isOutput=False)
    b1p_d = nc.declare_dram_parameter("B1p", [128, 2], f32, isOutput=False)
    a2_d = nc.declare_dram_parameter("A2", [128, 2], f32, isOutput=False)
    b2p_d = nc.declare_dram_parameter("B2p", [128, 2], f32, isOutput=False)
    saw_d = nc.declare_dram_parameter("saw", [128, S], f32, isOutput=False)
    sab_d = nc.declare_dram_parameter("sab", [128, S], f32, isOutput=False)
    outd = nc.declare_dram_parameter("out", [S, C, N], f32, isOutput=True)

    NCH = [(0, 128), (128, 68)]  # token-dim chunks of 196

    with ExitStack() as ctx:
        tc = ctx.enter_context(tile.TileContext(nc))
        wpool = ctx.enter_context(tc.tile_pool(name="wpool", bufs=1))
        xpool = ctx.enter_context(tc.tile_pool(name="xpool", bufs=3))
        qkpool = ctx.enter_context(tc.tile_pool(name="qkpool", bufs=2))
        vpool = ctx.enter_context(tc.tile_pool(name="vpool", bufs=2))
        epool = ctx.enter_context(tc.tile_pool(name="epool", bufs=2))
        opool = ctx.enter_context(tc.tile_pool(name="opool", bufs=2))
        otpool = ctx.enter_context(tc.tile_pool(name="otpool", bufs=2))
        t1pool = ctx.enter_context(tc.tile_pool(name="t1pool", bufs=2))
        hpool = ctx.enter_context(tc.tile_pool(name="hpool", bufs=3))
        tmppool = ctx.enter_context(tc.tile_pool(name="tmppool", bufs=3))
        gpool = ctx.enter_context(tc.tile_pool(name="gpool", bufs=3))
        outpool = ctx.enter_context(tc.tile_pool(name="outpool", bufs=2))
        smpool = ctx.enter_context(tc.tile_pool(name="smpool", bufs=3))

        psmm = ctx.enter_context(tc.tile_pool(name="psmm", bufs=2, space="PSUM"))
        pssc = ctx.enter_context(tc.tile_pool(name="pssc", bufs=2, space="PSUM"))
        psou = ctx.enter_context(tc.tile_pool(name="psou", bufs=2, space="PSUM"))
        psf = ctx.enter_context(tc.tile_pool(name="psf", bufs=1, space="PSUM"))

        # ---- resident weights / params ----
        wq_sb = wpool.tile([128, 2, C], bf)
        wk_sb = wpool.tile([128, 2, C], bf)
        wv_sb = wpool.tile([128, 2, C], bf)
        wp_sb = wpool.tile([128, 2, C], bf)
        w1_sb = wpool.tile([128, 2, CM], bf)
        w2_sb = wpool.tile([128, 16, C], bf)
        id_sb = wpool.tile([128, 128], bf)
        bq_sb = wpool.tile([128, 2], f32)
        bk_sb = wpool.tile([128, 2], f32)
        b1_sb = wpool.tile([128, 16], f32)
        a1_sb = wpool.tile([128, 2], f32)
        b1p_sb = wpool.tile([128, 2], f32)
        a2_sb = wpool.tile([128, 2], f32)
        b2p_sb = wpool.tile([128, 2], f32)
        saw_sb = wpool.tile([128, S], f32)
        sab_sb = wpool.tile([128, S], f32)

        for sb, dr in (
            (wq_sb, wq_d), (wk_sb, wk_d), (wv_sb, wv_d), (wp_sb, wp_d),
        ):
            nc.sync.dma_start(out=sb, in_=dr.rearrange("(cc p) j -> p cc j", p=128))
        nc.sync.dma_start(out=w1_sb, in_=w1_d.rearrange("(cc p) j -> p cc j", p=128))
        nc.sync.dma_start(out=w2_sb, in_=w2_d.rearrange("(kc p) j -> p kc j", p=128))
        for sb, dr in (
            (id_sb, id_d), (bq_sb, bq_d), (bk_sb, bk_d), (b1_sb, b1_d),
            (a1_sb, a1_d), (b1p_sb, b1p_d), (a2_sb, a2_d), (b2p_sb, b2p_d),
            (saw_sb, saw_d), (sab_sb, sab_d),
        ):
            nc.sync.dma_start(out=sb, in_=dr.ap())

        # ---- software-pipelined per-pair emission ----
        # attn(p+1) is emitted as a generator whose chunks are interleaved
        # between ffn(p)'s slabs, so the PE always has independent matmul
        # work queued and the HAM clock gate stays open.

        def emit_attn(pi):
            s0 = 2 * pi
            xt = xpool.tile([128, 2, 2, N], bf, name=f"xt{pi}", tag="xt")  # [p, cc, s2, n]
            for s2 in range(2):
                nc.sync.dma_start(
                    out=xt[:, :, s2],
                    in_=xd[s0 + s2].rearrange("(cc p) n -> p cc n", p=128),
                )

            # q^T, k^T : [p, ch(2 chunks of 128), s2, n]; head h = 4*ch+hp
            # lives at partition base 32*hp of chunk ch.
            qt = qkpool.tile([128, 2, 2, N], bf, name=f"qt{pi}", tag="qt")
            kt = qkpool.tile([128, 2, 2, N], bf, name=f"kt{pi}", tag="kt")
            for wt, bt, dst in ((wq_sb, bq_sb, qt), (wk_sb, bk_sb, kt)):
                for ch in range(2):
                    ps1 = psmm.tile([128, 2, N], f32, name=f"psqk{pi}_{ch}", tag="mm")
                    for cc in range(2):
                        nc.tensor.matmul(
                            ps1, wt[:, cc, ch * 128:(ch + 1) * 128], xt[:, cc],
                            start=(cc == 0), stop=(cc == 1),
                        )
                    nc.scalar.activation(
                        dst[:, ch], ps1, AF.Identity, bias=bt[:, ch:ch + 1]
                    )

            # v (token-major, 33-strided heads with ones column)
            vt = vpool.tile([128, 2, 2, HEADS * 33], bf, name=f"vt{pi}", tag="vt")
            for s2 in range(2):
                for mc, (n0, nsz) in enumerate(NCH):
                    psv = psmm.tile([128, 392], f32, name=f"psv{pi}_{s2}_{mc}", tag="mm")
                    for cc in range(2):
                        nc.tensor.matmul(
                            psv[:nsz, :C], xt[:, cc, s2, n0:n0 + nsz], wv_sb[:, cc],
                            start=(cc == 0), stop=(cc == 1),
                        )
                    vv = vt[:nsz, s2, mc].rearrange("p (h e) -> p h e", e=33)
                    nc.vector.tensor_copy(
                        vv[:, :, 0:32],
                        psv[:nsz, :C].rearrange("p (h d) -> p h d", d=32),
                    )
                    nc.gpsimd.memset(vv[:, :, 32:33], 1.0)
            yield None

            # scores^T + exp: one psum tile per (ch, hp, mc); the 4 hp row
            # groups of the PE array run concurrently via tile_position.
            expt = epool.tile([128, 2, HEADS, 2, N], bf, name=f"expt{pi}", tag="ex")
            for ch in range(2):
                for hp in range(4):
                    h = 4 * ch + hp
                    for mc, (m0, msz) in enumerate(NCH):
                        pss = pssc.tile(
                            [128, 2, N], f32, name=f"pss{pi}_{ch}_{hp}_{mc}", tag="sc"
                        )
                        for s2 in range(2):
                            nc.tensor.matmul(
                                pss[:msz, s2],
                                kt[32 * hp:32 * hp + 32, ch, s2, m0:m0 + msz],
                                qt[32 * hp:32 * hp + 32, ch, s2],
                                start=True, stop=True,
                                tile_position=(32 * hp, 0),
                            )
                        nc.scalar.activation(expt[:msz, mc, h], pss[:msz], AF.Exp)
                yield None

            # o = attn @ v (colsum via ones col), normalize on evac
            o_sb = opool.tile([128, 2, 2, C], bf, name=f"o{pi}", tag="o")
            for s2 in range(2):
                for ni, (n0, nsz) in enumerate(NCH):
                    pso = psou.tile([128, 392], f32, name=f"pso{pi}_{s2}_{ni}", tag="ou")
                    for h in range(HEADS):
                        for mc, (m0, msz) in enumerate(NCH):
                            nc.tensor.matmul(
                                pso[:nsz, h * 33:(h + 1) * 33],
                                expt[:msz, mc, h, s2, n0:n0 + nsz],
                                vt[:msz, s2, mc, h * 33:(h + 1) * 33],
                                start=(mc == 0), stop=(mc == 1),
                            )
                    pv = pso[:nsz, :HEADS * 33].rearrange("p (h e) -> p h e", e=33)
                    rc = smpool.tile([128, HEADS], f32, name=f"rc{pi}_{s2}_{ni}", tag="rc")
                    nc.vector.reciprocal(rc[:nsz], pv[:, :, 32])
                    ob = o_sb[:nsz, s2, ni].rearrange("p (h d) -> p h d", d=32)
                    rcb = bass.AP(
                        tensor=rc.tensor, offset=rc.offset,
                        ap=[list(rc.ap[0])[:1] + [nsz]] + [list(rc.ap[1])] + [[0, 32]],
                    )
                    nc.vector.tensor_tensor(ob, pv[:, :, 0:32], rcb, OP.mult)
            yield None

            # transpose o -> oT [c, n]
            oT_sb = otpool.tile([128, 2, 2, N], bf, name=f"oT{pi}", tag="oT")
            for cc in range(2):
                psot = pssc.tile([128, 2, 392], bf, name=f"psot{pi}_{cc}", tag="sc")
                for s2 in range(2):
                    for ni, (n0, nsz) in enumerate(NCH):
                        nc.tensor.transpose(
                            psot[:, s2, n0:n0 + nsz],
                            o_sb[:nsz, s2, ni, cc * 128:(cc + 1) * 128],
                            id_sb[:nsz, :nsz],
                        )
                nc.vector.tensor_copy(oT_sb[:, cc], psot[:, :, :N])

            # proj + residual + RepBN1:  t1 = A1*psp + (A1*xt + B1p)
            xb = tmppool.tile([128, 2, 2, N], bf, name=f"xb{pi}", tag="xb")
            for jc in range(2):
                nc.vector.tensor_scalar(
                    xb[:, jc], xt[:, jc], a1_sb[:, jc:jc + 1], b1p_sb[:, jc:jc + 1],
                    OP.mult, OP.add,
                )
            t1_sb = t1pool.tile([128, 2, 2, N], bf, name=f"t1{pi}", tag="t1")
            for jc in range(2):
                psp = pssc.tile([128, 2, N], f32, name=f"psp{pi}_{jc}", tag="sc")
                for cc in range(2):
                    nc.tensor.matmul(
                        psp, wp_sb[:, cc, jc * 128:(jc + 1) * 128], oT_sb[:, cc],
                        start=(cc == 0), stop=(cc == 1),
                    )
                nc.vector.scalar_tensor_tensor(
                    t1_sb[:, jc], psp, a1_sb[:, jc:jc + 1], xb[:, jc],
                    OP.mult, OP.add,
                )
            yield t1_sb

        def emit_ffn(pi, t1_sb, gen=None):
            s0 = 2 * pi
            psfj = [
                psf.tile([128, 2, N], f32, name=f"psf{pi}_{jc}", tag=f"f{jc}")
                for jc in range(2)
            ]
            # t1b = A2*t1 + B2p, consumed by the fused output evac
            t1b = tmppool.tile([128, 2, 2, N], bf, name=f"t1b{pi}", tag="t1b")
            for jc in range(2):
                nc.vector.tensor_scalar(
                    t1b[:, jc], t1_sb[:, jc], a2_sb[:, jc:jc + 1],
                    b2p_sb[:, jc:jc + 1], OP.mult, OP.add,
                )
            # 4-kc slabs; SILU via tanh (same ACT table set as exp):
            #   u = (h+c)*h ; th = tanh(saw2*u) ; g' = (th+1)*h ; W2 pre-halved
            for sl in range(4):
                h_all = hpool.tile([128, 4, 2, N], bf, name=f"hs{pi}_{sl}", tag="hs")
                for k4 in range(4):
                    kc = sl * 4 + k4
                    psh = psmm.tile([128, 2, N], f32, name=f"psh{pi}_{kc}", tag="mm")
                    for cc in range(2):
                        nc.tensor.matmul(
                            psh, w1_sb[:, cc, kc * 128:(kc + 1) * 128], t1_sb[:, cc],
                            start=(cc == 0), stop=(cc == 1),
                        )
                    if kc % 4 == 0:
                        nc.vector.tensor_scalar(
                            h_all[:, k4], psh, b1_sb[:, kc:kc + 1], None, OP.add
                        )
                    else:
                        nc.scalar.activation(
                            h_all[:, k4], psh, AF.Identity, bias=b1_sb[:, kc:kc + 1]
                        )
                u = tmppool.tile([128, 4, 2, N], bf, name=f"u{pi}_{sl}", tag="u")
                th = tmppool.tile([128, 4, 2, N], bf, name=f"th{pi}_{sl}", tag="th")
                g_all = gpool.tile([128, 4, 2, N], bf, name=f"g{pi}_{sl}", tag="g")
                if has_c:
                    for s2 in range(2):
                        nc.vector.scalar_tensor_tensor(
                            u[:, :, s2], h_all[:, :, s2],
                            sab_sb[:, s0 + s2:s0 + s2 + 1], h_all[:, :, s2],
                            OP.add, OP.mult,
                        )
                else:
                    nc.vector.tensor_tensor(u, h_all, h_all, OP.mult)
                for s2 in range(2):
                    nc.scalar.activation(
                        th[:, :, s2], u[:, :, s2], AF.Tanh,
                        scale=saw_sb[:, s0 + s2:s0 + s2 + 1],
                    )
                nc.vector.scalar_tensor_tensor(
                    g_all, th, 1.0, h_all, OP.add, OP.mult,
                )
                for k4 in range(4):
                    kc = sl * 4 + k4
                    for jc in range(2):
                        nc.tensor.matmul(
                            psfj[jc], w2_sb[:, kc, jc * 128:(jc + 1) * 128],
                            g_all[:, k4],
                            start=(kc == 0), stop=(kc == 15),
                        )
                if gen is not None:
                    next(gen, None)

            # residual + RepBN2 -> out:  ot = A2*psf + t1b
            ot = outpool.tile([128, 2, 2, N], f32, name=f"ot{pi}", tag="ot")
            for jc in range(2):
                nc.vector.scalar_tensor_tensor(
                    ot[:, jc], psfj[jc], a2_sb[:, jc:jc + 1], t1b[:, jc],
                    OP.mult, OP.add,
                )
            for s2 in range(2):
                nc.sync.dma_start(
                    out=outd[s0 + s2].rearrange("(cc p) n -> p cc n", p=128),
                    in_=ot[:, :, s2],
                )

        def drain(g):
            t1 = None
            for v in g:
                if v is not None:
                    t1 = v
            return t1

        npairs = S // 2
        t1_prev = drain(emit_attn(0))
        for pi in range(npairs):
            g = emit_attn(pi + 1) if pi + 1 < npairs else None
            emit_ffn(pi, t1_prev, g)
            t1_prev = drain(g) if g is not None else None

    return nc


def _get_nc(S, has_c=False):
    key = (S, has_c)
    if key not in _NC_CACHE:
        _NC_CACHE[key] = _build(S, has_c)
    return _NC_CACHE[key]


def _prep_inputs(inputs, S):
    """Host-side preprocessing + sharding. Returns in_maps (len NCORES)."""
    x = np.asarray(inputs["x"], np.float32).reshape(B, C, N)
    W_qkv = np.asarray(inputs["W_qkv"], np.float32)
    b_qkv = np.asarray(inputs["b_qkv"], np.float32)
    W_proj = np.asarray(inputs["W_proj"], np.float32)
    b_proj = np.asarray(inputs["b_proj"], np.float32)
    W1 = np.asarray(inputs["W1"], np.float32)
    b1 = np.asarray(inputs["b1"], np.float32)
    W2 = np.asarray(inputs["W2"], np.float32)
    b2 = np.asarray(inputs["b2"], np.float32)
    sa_w = np.asarray(inputs["sa_w"], np.float32)
    sa_b = np.asarray(inputs["sa_b"], np.float32)

    def g(name):
        return np.asarray(inputs[name], np.float32)

    scale = D ** -0.5
    Wq = W_qkv[:, 0:C] * scale
    bq = b_qkv[0:C] * scale
    Wk = W_qkv[:, C:2 * C]
    bk = b_qkv[C:2 * C]
    Wv = W_qkv[:, 2 * C:3 * C]
    bv = b_qkv[2 * C:3 * C]
    b_proj_eff = b_proj + bv @ W_proj

    s1 = g("gamma1") / np.sqrt(g("rv1") + EPS)
    A1 = np.float32(g("alpha1")) + s1
    B1 = g("beta1") - g("rm1") * s1
    B1p = A1 * b_proj_eff + B1
    s2v = g("gamma2") / np.sqrt(g("rv2") + EPS)
    A2 = np.float32(g("alpha2")) + s2v
    B2 = g("beta2") - g("rm2") * s2v
    B2p = A2 * b2 + B2

    # sigmoid(w) = (1+tanh(w/2))/2 : tanh lives in the same ACT table set as
    # exp, so the kernel never reloads activation tables. Fold the /2 into
    # sa_w (tanh arg) and W2 (output):  g = h*sigmoid((sa_w*h+sa_b)*h)
    #   u  = (h + c)*h          with c = sa_b/sa_w
    #   th = tanh(saw2 * u)     with saw2 = sa_w/2   (ACT scale)
    #   g' = (th + 1)*h ;  f = g' @ (W2/2)
    saw2 = sa_w * 0.5
    c_silu = np.divide(sa_b, sa_w, out=np.zeros_like(sa_b),
                       where=(sa_w != 0)).astype(np.float32)

    def v2sb(v):  # [256] -> [128, 2]
        return np.ascontiguousarray(v.reshape(2, 128).T)

    def v2sb96(v):  # [256] -> [128, 4], chunks of 64 zero-padded
        o = np.zeros((128, 4), np.float32)
        for ch in range(4):
            o[0:64, ch] = v[ch * 64:(ch + 1) * 64]
        return o

    common = {
        "wq": Wq.astype(BF16), "wk": Wk.astype(BF16), "wv": Wv.astype(BF16),
        "wp": W_proj.astype(BF16), "w1": W1.astype(BF16),
        "w2": (W2 * 0.5).astype(BF16),
        "ident": np.eye(128, dtype=BF16),
        "bq": v2sb(bq), "bk": v2sb(bk),
        "b1": np.ascontiguousarray(b1.reshape(16, 128).T),
        "A1": v2sb(A1), "B1p": v2sb(B1p), "A2": v2sb(A2), "B2p": v2sb(B2p),
    }
    xb = x.astype(BF16)
    in_maps = []
    for c in range(NCORES):
        sl = slice(c * S, (c + 1) * S)
        m = dict(common)
        m["x"] = np.ascontiguousarray(xb[sl])
        m["saw"] = np.ascontiguousarray(
            np.broadcast_to(saw2[sl][None, :], (128, S)).astype(np.float32))
        m["sab"] = np.ascontiguousarray(
            np.broadcast_to(c_silu[sl][None, :], (128, S)).astype(np.float32))
        in_maps.append(m)
    return in_maps


_LDW_OPT = [False]


def _patch_ldw_opt():
    if _LDW_OPT[0]:
        return
    import concourse.bass_utils as bu
    orig = bu.run_command

    def patched(cmd, **kw):
        cmd = ["--enable-ldw-opt=true" if c == "--enable-ldw-opt=false" else c
               for c in cmd]
        return orig(cmd, **kw)

    bu.run_command = patched
    _LDW_OPT[0] = True


def run(trace=False, **inputs):
    """Returns (out [B,C,H,W] f32, exec_time_ns or None)."""
    from concourse.bass_utils import run_bass_kernel_spmd

    S = B // NCORES
    has_c = bool(np.any(np.asarray(inputs["sa_b"], np.float32) != 0))
    nc = _get_nc(S, has_c)
    if not nc.is_finalized():
        nc.finalize()
    in_maps = _prep_inputs(inputs, S)
    res = run_bass_kernel_spmd(nc, in_maps, core_ids=list(range(NCORES)), trace=trace)
    outs = [np.asarray(r["out"], np.float32) for r in res.results]
    out = np.concatenate(outs, axis=0).reshape(B, C, HH, WW)
    return out, res.exec_time_ns


def kernel(**inputs):
    return run(trace=False, **inputs)[0]

